# revision 1
# baseline (speedup 1.0000x reference)
"""Trainium2 Bass kernel for nn_BaseAttention (B=2, N=2048, E=2048, H=16, D=128).

Sharding: 8 cores; core c handles batch b=c//4, head-group hg=c%4 (4 heads).
Each core computes q/k/v projections for its heads, causal flash-style
attention, and a partial out-projection (contraction over its 512 head dims).
Host sums the 4 partial outputs per batch (tensor-parallel unshard).

Projections and out-projection run as float32r (full PE rate at free dim >=
256). q/k spill to DRAM as bf16, so QK^T and A@V are bf16 matmuls. exp runs
on ScalarE straight out of PSUM (only the causally-valid region); softmax
denominators use a DVE pairwise tree + ones-matmul partition reduction, a
K=1 broadcast matmul, and the fast approximate reciprocal.
"""

import os
import sys
import time

sys.path.insert(0, "/opt/trn_rl_repo")

PHASES = {"proj", "attn", "norm", "oproj"}

import numpy as np
import ml_dtypes

import concourse.bass as bass
import concourse.mybir as mybir
import concourse.tile as tile
from concourse import bacc
from concourse.bass_utils import run_bass_kernel_spmd

B, N, E, H = 2, 2048, 2048, 16
D = E // H            # 128
HPC = 4               # heads per core
DC = HPC * D          # 512 head dims per core
NCORES = 8
P = 128
NCH = N // 512        # 4 n-chunks of 512
ET = E // P           # 16 e-tiles of 128

F32 = mybir.dt.float32
F32R = mybir.dt.float32r
BF16 = mybir.dt.bfloat16
FP16 = mybir.dt.float16


def build_nc():
    nc = bacc.Bacc("TRN2", target_bir_lowering=False, debug=False,
                   num_devices=NCORES)

    xT = nc.dram_tensor("xT", [E, N], FP16, kind="ExternalInput")
    wqT = nc.dram_tensor("wqT", [E, DC], FP16, kind="ExternalInput")
    wkT = nc.dram_tensor("wkT", [E, DC], FP16, kind="ExternalInput")
    wvT = nc.dram_tensor("wvT", [E, DC], FP16, kind="ExternalInput")
    woT = nc.dram_tensor("woT", [DC, E], FP16, kind="ExternalInput")
    maskin = nc.dram_tensor("maskin", [P, 4, 512], FP16, kind="ExternalInput")
    out = nc.dram_tensor("out", [N, E], F32, kind="ExternalOutput")

    xT_r = xT.ap().rearrange("(eo p) n -> p eo n", p=P)      # [128,16,2048]
    wqT_r = wqT.ap().rearrange("(eo p) d -> p eo d", p=P)    # [128,16,512]
    wkT_r = wkT.ap().rearrange("(eo p) d -> p eo d", p=P)
    wvT_r = wvT.ap().rearrange("(eo p) d -> p eo d", p=P)
    woT_r = woT.ap().rearrange("(t p) e -> p t e", p=P)      # [128,4,2048]

    with tile.TileContext(nc) as tc:
        # ---------------- constants + spill tensors ----------------
        consts = tc.alloc_tile_pool(name="consts", bufs=1)
        _longlived = [consts]
        mask_sb = consts.tile([P, 4, 512], FP16)
        # prefire the Exp table load so it overlaps the input DMA head
        dummy = consts.tile([1, 8], F32)
        nc.vector.memset(dummy, 0.0)
        nc.scalar.activation(out=dummy, in_=dummy,
                             func=mybir.ActivationFunctionType.Exp)

        dram = tc.alloc_tile_pool(name="dram", bufs=1, space="DRAM")
        _longlived.append(dram)
        attd = dram.tile([HPC, N, D], FP16)          # normalized attn out

        # per-core activations, SBUF-resident across the whole kernel
        big = tc.alloc_tile_pool(name="big", bufs=1)
        _longlived.append(big)
        qs = big.tile([P, HPC, N], FP16)                  # q^T, heads stacked
        ks = big.tile([P, HPC, N], FP16)                  # k^T
        v_all = big.tile([P, N // P, HPC, D + 4], FP16)   # [V | 1] per block
        nc.vector.memset(v_all[:, :, :, D:D + 1], 1.0)

        # ---------------- phase 1: q/k/v projections ----------------
        if "proj" in PHASES:
         with (
            tc.tile_pool(name="wpool", bufs=1) as wpool,
            tc.tile_pool(name="xpool", bufs=2) as xpool,
            tc.tile_pool(name="pj_ps", bufs=4, space="PSUM") as pj_ps,
         ):
            wq_sb = wpool.tile([P, ET, DC], FP16)
            wk_sb = wpool.tile([P, ET, DC], FP16)
            wv_sb = wpool.tile([P, ET, DC], FP16)
            x_tiles = [None] * NCH

            def load_x(nch):
                t = xpool.tile([P, ET, 512], FP16, tag="xchunk",
                               name=f"x_sb{nch}")
                nc.sync.dma_start(
                    out=t, in_=xT_r[:, :, nch * 512:(nch + 1) * 512])
                x_tiles[nch] = t

            # interleaved preload in 1MB pieces: PE starts after the first
            # wq piece + x0 piece; DMA then feeds just-in-time
            QT = ET // 4
            x0 = xpool.tile([P, ET, 512], FP16, tag="xchunk", name="x_sb0")
            x_tiles[0] = x0
            # extra-fine first pieces so the first matmul starts earlier
            for g2, (a, b) in enumerate(((0, 1), (1, 2), (2, 4))):
                gs = slice(a, b)
                nc.sync.dma_start(out=wq_sb[:, gs, :], in_=wqT_r[:, gs, :])
                nc.sync.dma_start(out=x0[:, gs, :],
                                  in_=xT_r[:, gs, 0:512])
            for g in range(1, 4):
                gs = slice(g * QT, (g + 1) * QT)
                nc.sync.dma_start(out=wq_sb[:, gs, :], in_=wqT_r[:, gs, :])
                nc.sync.dma_start(out=x0[:, gs, :],
                                  in_=xT_r[:, gs, 0:512])
            for g in range(4):
                gs = slice(g * QT, (g + 1) * QT)
                nc.sync.dma_start(out=wk_sb[:, gs, :], in_=wkT_r[:, gs, :])
            load_x(1)
            nc.sync.dma_start(out=mask_sb, in_=maskin.ap())
            HF = ET // 2
            nc.sync.dma_start(out=wv_sb[:, :HF, :], in_=wvT_r[:, :HF, :])
            nc.sync.dma_start(out=wv_sb[:, HF:, :], in_=wvT_r[:, HF:, :])

            for nch in range(NCH):
                x_sb = x_tiles[nch]
                nsl = slice(nch * 512, (nch + 1) * 512)

                # qT / kT: psum[dq_tile 128, 512 n] = sum_e W[e, dq] x[e, n]
                for w_sb, dst in ((wq_sb, qs), (wk_sb, ks)):
                    for t in range(HPC):
                        ps = pj_ps.tile([P, 512], F32, tag="pjps")
                        for et in range(ET):
                            nc.tensor.matmul(
                                ps,
                                lhsT=w_sb[:, et, t * P:(t + 1) * P],
                                rhs=x_sb[:, et, :],
                                start=(et == 0), stop=(et == ET - 1),
                            )
                        if nch == NCH - 1:
                            nc.scalar.copy(out=dst[:, t, nsl], in_=ps)
                        else:
                            nc.vector.tensor_copy(out=dst[:, t, nsl], in_=ps)

                # v: psum[n_block 128, 512 dv] = sum_e x[e, n] Wv[e, dv]
                for nb in range(4):
                    ps = pj_ps.tile([P, 512], F32, tag="pjps")
                    for et in range(ET):
                        nc.tensor.matmul(
                            ps,
                            lhsT=x_sb[:, et, nb * P:(nb + 1) * P],
                            rhs=wv_sb[:, et, :],
                            start=(et == 0), stop=(et == ET - 1),
                        )
                    if nch == NCH - 1:
                        nc.scalar.copy(
                            out=v_all[:, nch * 4 + nb, :, :D],
                            in_=ps.rearrange("p (h d) -> p h d", h=HPC))
                    else:
                        nc.vector.tensor_copy(
                            out=v_all[:, nch * 4 + nb, :, :D],
                            in_=ps.rearrange("p (h d) -> p h d", h=HPC))

                if nch + 2 < NCH:
                    load_x(nch + 2)

        # ---------------- phase 2: attention ----------------
        outT_pool = tc.alloc_tile_pool(name="outT", bufs=1)
        _longlived.append(outT_pool)
        outTs = [outT_pool.tile([P, N], FP16, name=f"outT{t}")
                 for t in range(HPC)]
        wo_pool = tc.alloc_tile_pool(name="wo_pool", bufs=1)
        _longlived.append(wo_pool)
        wo_sb = wo_pool.tile([P, HPC, E], FP16)
        if "oproj" in PHASES and "attn" not in PHASES:
            for t in range(HPC):
                nc.sync.dma_start(out=wo_sb[:, t, :], in_=woT_r[:, t, :])

        if "attn" in PHASES:
         with (
            tc.tile_pool(name="pt_pool", bufs=3) as pt_pool,
            tc.tile_pool(name="att_pool", bufs=3) as att_pool,
            tc.tile_pool(name="rs_pool", bufs=8) as rs_pool,
            tc.tile_pool(name="qk_ps", bufs=2, space="PSUM") as qk_ps,
            tc.tile_pool(name="av_ps", bufs=4, space="PSUM") as av_ps,
         ):
            for h in range(HPC):
                if "oproj" in PHASES:
                    nc.sync.dma_start(out=wo_sb[:, h, :], in_=woT_r[:, h, :])
                att_h = att_pool.tile([P, N // P, D], FP16, tag="atth")

                for ci in range(NCH):
                    BJ = 4 * (ci + 1)
                    pt = pt_pool.tile([P, ET, 512], FP16, tag="pt")
                    if h == 0:
                        # first use of each slice range of the rotating pool:
                        # clear the regions partial-exp never writes so the
                        # mask multiply sees finite values, not NaN garbage
                        nc.vector.memset(pt[:, BJ - 2, :256], 0.0)
                        nc.vector.memset(pt[:, BJ - 1, :384], 0.0)
                    # scores^T tiles [j_block, i_chunk] + exp (2 tiles/ACT op)
                    for bjp in range(BJ // 2):
                        ps = qk_ps.tile([P, 2, 512], F32, tag="qkps")
                        last_pair = (bjp == BJ // 2 - 1)
                        for u in range(2):
                            bj = 2 * bjp + u
                            nc.tensor.matmul(
                                ps[:, u, :],
                                lhsT=ks[:, h, bj * P:(bj + 1) * P],
                                rhs=qs[:, h, ci * 512:(ci + 1) * 512],
                                start=True, stop=True,
                            )
                        if last_pair:
                            # diagonal blocks r=256,384: only cols >= r valid
                            nc.scalar.activation(
                                out=pt[:, 2 * bjp, 256:], in_=ps[:, 0, 256:],
                                func=mybir.ActivationFunctionType.Exp,
                            )
                            nc.scalar.activation(
                                out=pt[:, 2 * bjp + 1, 384:], in_=ps[:, 1, 384:],
                                func=mybir.ActivationFunctionType.Exp,
                            )
                        else:
                            nc.scalar.activation(
                                out=pt[:, 2 * bjp:2 * bjp + 2, :], in_=ps,
                                func=mybir.ActivationFunctionType.Exp,
                            )
                    # causal masks on the diagonal blocks (bj = BJ-4 .. BJ-1)
                    # full-tile: the mask's zero prefix also clears regions
                    # exp never wrote (stale finite values from pool reuse)
                    for rr in range(4):
                        bj = BJ - 4 + rr
                        nc.vector.tensor_mul(
                            out=pt[:, bj, :], in0=pt[:, bj, :],
                            in1=mask_sb[:, rr, :])

                    # A @ [V | 1]: out rows are queries, col 128 is the
                    # softmax denominator; normalize on eviction
                    for ib in range(4):
                        avp = av_ps.tile([P, D + 4], F32, tag="avps")
                        isl = slice(ib * P, (ib + 1) * P)
                        for bj in range(BJ):
                            nc.tensor.matmul(
                                avp[:, :D + 1],
                                lhsT=pt[:, bj, isl],
                                rhs=v_all[:, bj, h, :D + 1],
                                start=(bj == 0), stop=(bj == BJ - 1),
                            )
                        rs = rs_pool.tile([P, 1], F32, tag="rs")
                        nc.vector.reciprocal_approx_fast(
                            out=rs, in_=avp[:, D:D + 1])
                        nc.vector.tensor_scalar_mul(
                            out=att_h[:, ci * 4 + ib, :], in0=avp[:, :D],
                            scalar1=rs)

                    # spill + transpose this ci's slice right away so the
                    # out-projection isn't gated on the whole head
                    csl = slice(ci * 4, (ci + 1) * 4)
                    nc.sync.dma_start(
                        out=attd[h, ci * 512:(ci + 1) * 512, :].rearrange(
                            "(io p) d -> p io d", p=P),
                        in_=att_h[:, csl, :])
                    nc.sync.dma_start_transpose(
                        out=outTs[h][:, ci * 512:(ci + 1) * 512],
                        in_=attd[h, ci * 512:(ci + 1) * 512, :])

        # ---------------- phase 4: out projection (partial) ----------------
        if "oproj" in PHASES:
         with (
            tc.tile_pool(name="op_ps", bufs=4, space="PSUM") as op_ps,
            tc.tile_pool(name="op_ev", bufs=3) as op_ev,
         ):
            for nb in range(N // P):
                ostage = op_ev.tile([P, NCH, 512], F32, tag="opev")
                for ec in range(NCH):
                    ps = op_ps.tile([P, 512], F32, tag="opps")
                    for t in range(HPC):
                        nc.tensor.matmul(
                            ps,
                            lhsT=outTs[t][:, nb * P:(nb + 1) * P],
                            rhs=wo_sb[:, t, ec * 512:(ec + 1) * 512],
                            start=(t == 0), stop=(t == HPC - 1),
                        )
                    nc.any.tensor_copy(out=ostage[:, ec, :], in_=ps)
                nc.sync.dma_start(
                    out=out.ap()[nb * P:(nb + 1) * P, :], in_=ostage)

        for _pl in reversed(_longlived):
            _pl.release()

    nc.compile()
    return nc


def make_in_maps(x, Wq, Wkv, Wout):
    x = np.asarray(x, dtype=np.float32)
    Wq = np.asarray(Wq, dtype=np.float32)
    Wkv = np.asarray(Wkv, dtype=np.float32)
    Wout = np.asarray(Wout, dtype=np.float32)
    scale = np.float32(D ** -0.5)

    # causal masks for the 4 diagonal offsets
    jj = np.arange(P)[:, None]
    ii = np.arange(512)[None, :]
    mask = np.zeros((P, 4, 512), dtype=np.float16)
    for rr in range(4):
        mask[:, rr, :] = (ii >= jj + rr * P).astype(np.float16)

    xT = [np.ascontiguousarray(x[b].T).astype(np.float16) for b in range(B)]
    in_maps = []
    for c in range(NCORES):
        b, hg = divmod(c, 4)
        sl = slice(hg * DC, (hg + 1) * DC)
        in_maps.append({
            "xT": xT[b],
            "wqT": (np.ascontiguousarray(Wq[sl, :].T) * scale).astype(np.float16),
            "wkT": np.ascontiguousarray(Wkv[sl, :].T).astype(np.float16),
            "wvT": np.ascontiguousarray(Wkv[E + sl.start:E + sl.stop, :].T).astype(np.float16),
            "woT": np.ascontiguousarray(Wout[:, sl].T).astype(np.float16),
            "maskin": mask,
        })
    return in_maps


_NC_CACHE = []


def _get_nc():
    if not _NC_CACHE:
        _NC_CACHE.append(build_nc())
    return _NC_CACHE[0]


def _run(in_maps):
    nc = _get_nc()
    return run_bass_kernel_spmd(nc, in_maps, core_ids=list(range(NCORES)))


def kernel(x, Wq, Wkv, Wout):
    in_maps = make_in_maps(x, Wq, Wkv, Wout)
    res = _run(in_maps)
    out = np.zeros((B, N, E), dtype=np.float32)
    for c in range(NCORES):
        out[c // 4] += res.results[c]["out"]
    return out


if __name__ == "__main__":
    t0 = time.time()
    _get_nc()
    print(f"build+compile: {time.time() - t0:.1f}s")



# revision 23
# speedup vs baseline: 1.0668x; 1.0668x over previous
"""Trainium2 Bass kernel for nn_BaseAttention (B=2, N=2048, E=2048, H=16, D=128).

Sharding: 8 cores; core c handles batch b=c//4, head-group hg=c%4 (4 heads).
Each core computes q/k/v projections for its heads, causal flash-style
attention, and a partial out-projection (contraction over its 512 head dims).
Host sums the 4 partial outputs per batch (tensor-parallel unshard).

All matmuls fp16 (1 row/cycle at 2.4 GHz). Causal structure is exploited at
128-row granularity: diagonal QK^T blocks stream only causally-valid rows and
A@V skips fully-masked j-blocks. Attention for n-chunk c is interleaved into
projection chunk c+1 as micro-ops paced between PSUM groups, so ScalarE's exp
(the second-largest engine load) hides under projection matmuls instead of
gating the PE; the attention tail similarly interleaves with the
out-projection. PSUM evictions round-robin over DVE/Pool, keeping ACT free
for exp. Output partials are emitted as fp16 (summed in f32 on host).
"""

import os
import sys
import time

sys.path.insert(0, "/opt/trn_rl_repo")

import numpy as np
import ml_dtypes

import concourse.bass as bass
import concourse.mybir as mybir
import concourse.tile as tile
from concourse import bacc
from concourse.bass_utils import run_bass_kernel_spmd

B, N, E, H = 2, 2048, 2048, 16
D = E // H            # 128
HPC = 4               # heads per core
DC = HPC * D          # 512 head dims per core
NCORES = 8
P = 128
NCH = N // 512        # 4 n-chunks of 512
ET = E // P           # 16 e-tiles of 128
NB = N // P           # 16 n-blocks of 128

F32 = mybir.dt.float32
FP16 = mybir.dt.float16


def build_nc():
    nc = bacc.Bacc("TRN2", target_bir_lowering=False, debug=False,
                   num_devices=NCORES)

    xT = nc.dram_tensor("xT", [E, N], FP16, kind="ExternalInput")
    wqT = nc.dram_tensor("wqT", [E, DC], FP16, kind="ExternalInput")
    wkT = nc.dram_tensor("wkT", [E, DC], FP16, kind="ExternalInput")
    wvT = nc.dram_tensor("wvT", [E, DC], FP16, kind="ExternalInput")
    woT = nc.dram_tensor("woT", [DC, E], FP16, kind="ExternalInput")
    maskin = nc.dram_tensor("maskin", [P, 4, 512], FP16, kind="ExternalInput")
    out = nc.dram_tensor("out", [N, E], FP16, kind="ExternalOutput")

    xT_r = xT.ap().rearrange("(eo p) n -> p eo n", p=P)      # [128,16,2048]
    wqT_r = wqT.ap().rearrange("(eo p) d -> p eo d", p=P)    # [128,16,512]
    wkT_r = wkT.ap().rearrange("(eo p) d -> p eo d", p=P)
    wvT_r = wvT.ap().rearrange("(eo p) d -> p eo d", p=P)
    woT_r = woT.ap().rearrange("(t p) e -> p t e", p=P)      # [128,4,2048]

    with tile.TileContext(nc) as tc:
        # ---------------- long-lived pools ----------------
        consts = tc.alloc_tile_pool(name="consts", bufs=1)
        mask_sb = consts.tile([P, 4, 512], FP16)
        # prefire the Exp table load so it overlaps the input DMA head
        dummy = consts.tile([1, 8], F32)
        nc.vector.memset(dummy, 0.0)
        nc.scalar.activation(out=dummy, in_=dummy,
                             func=mybir.ActivationFunctionType.Exp)

        dram = tc.alloc_tile_pool(name="dram", bufs=1, space="DRAM")
        attd = dram.tile([HPC, N, D], FP16)          # normalized attn out

        big = tc.alloc_tile_pool(name="big", bufs=1)
        qs = big.tile([P, HPC, N], FP16)                  # q^T, heads stacked
        ks = big.tile([P, HPC, N], FP16)                  # k^T
        v_all = big.tile([P, NB, HPC, D + 4], FP16)       # [V | 1] per block
        nc.vector.memset(v_all[:, :, :, D:D + 1], 1.0)

        outT_pool = tc.alloc_tile_pool(name="outT", bufs=1)
        outTs = [outT_pool.tile([P, N], FP16, name=f"outT{t}")
                 for t in range(HPC)]

        # ---------------- attention pools ----------------
        pt_pool = tc.alloc_tile_pool(name="pt_pool", bufs=3)
        att_pool = tc.alloc_tile_pool(name="att_pool", bufs=2)
        rs_pool = tc.alloc_tile_pool(name="rs_pool", bufs=8)
        qk_ps = tc.alloc_tile_pool(name="qk_ps", bufs=2, space="PSUM")
        av_ps = tc.alloc_tile_pool(name="av_ps", bufs=2, space="PSUM")
        # ---- proj-phase pools, on top of the stack (released before oproj)
        wpool = tc.alloc_tile_pool(name="wpool", bufs=1)
        xpool = tc.alloc_tile_pool(name="xpool", bufs=2)
        pj_ps = tc.alloc_tile_pool(name="pj_ps", bufs=2, space="PSUM")

        wq_sb = wpool.tile([P, ET, DC], FP16)
        wk_sb = wpool.tile([P, ET, DC], FP16)
        wv_sb = wpool.tile([P, ET, DC], FP16)
        x_tiles = [None] * NCH

        def load_x(nch):
            # on the SP queue: the Pool queue carries evictions, and this
            # DMA's WAR wait on the old buffer would head-of-line block them
            t = xpool.tile([P, ET, 512], FP16, tag="xchunk",
                           name=f"x_sb{nch}")
            nc.sync.dma_start(
                out=t, in_=xT_r[:, :, nch * 512:(nch + 1) * 512])
            x_tiles[nch] = t

        # two DMA queues (SP + Pool) with wq/x0 split in eo-pairs and issued
        # alternately: DMA transfers serialize globally in trigger order, so
        # arrival must match consumption: wq/x0 interleaved (first qk
        # groups), then wk, wv, mask, and x1 last (needed only at chunk 1)
        x0 = xpool.tile([P, ET, 512], FP16, tag="xchunk", name="x_sb0")
        x_tiles[0] = x0
        nc.sync.dma_start(out=wq_sb[:, 0:1, :], in_=wqT_r[:, 0:1, :])
        nc.gpsimd.dma_start(out=x0[:, 0:1, :], in_=xT_r[:, 0:1, 0:512])
        nc.sync.dma_start(out=wq_sb[:, 1:2, :], in_=wqT_r[:, 1:2, :])
        nc.gpsimd.dma_start(out=x0[:, 1:2, :], in_=xT_r[:, 1:2, 0:512])
        for g in range(1, ET // 2):
            gs = slice(2 * g, 2 * g + 2)
            nc.sync.dma_start(out=wq_sb[:, gs, :], in_=wqT_r[:, gs, :])
            nc.gpsimd.dma_start(out=x0[:, gs, :], in_=xT_r[:, gs, 0:512])
        for g in range(ET // 2):
            gs = slice(2 * g, 2 * g + 2)
            nc.sync.dma_start(out=wk_sb[:, gs, :], in_=wkT_r[:, gs, :])
        QT = ET // 4
        for g in range(4):
            gs = slice(g * QT, (g + 1) * QT)
            nc.gpsimd.dma_start(out=wv_sb[:, gs, :], in_=wvT_r[:, gs, :])
        nc.gpsimd.dma_start(out=mask_sb, in_=maskin.ap())
        x1 = xpool.tile([P, ET, 512], FP16, tag="xchunk", name="x_sb1")
        x_tiles[1] = x1
        nc.gpsimd.dma_start(out=x1[:, :ET // 2, :],
                            in_=xT_r[:, :ET // 2, 512:1024])
        nc.gpsimd.dma_start(out=x1[:, ET // 2:, :],
                            in_=xT_r[:, ET // 2:, 512:1024])

        # PSUM evictions: only DVE and ACT can read PSUM (GPSIMD cannot).
        # ACT is reserved for exp while attention is interleaved, so it only
        # helps during chunk 0 and the out-projection tail.
        _ev = [0]
        _copies = (nc.vector.tensor_copy, nc.scalar.copy)

        def evict(out_ap, in_ap, use_act):
            n = 2 if use_act else 1
            _copies[_ev[0] % n](out=out_ap, in_=in_ap)
            _ev[0] += 1

        # ---------------- proj psum-group closures ----------------
        def proj_groups(nch):
            x_sb = x_tiles[nch]
            nsl = slice(nch * 512, (nch + 1) * 512)
            groups = []

            def qk_group(w_sb, dst, t):
                def run():
                    ps = pj_ps.tile([P, 512], F32, tag="pjps")
                    for et in range(ET):
                        nc.tensor.matmul(
                            ps,
                            lhsT=w_sb[:, et, t * P:(t + 1) * P],
                            rhs=x_sb[:, et, :],
                            start=(et == 0), stop=(et == ET - 1),
                        )
                    evict(dst[:, t, nsl], ps, nch == 0)
                return run

            def v_group(nb):
                def run():
                    ps = pj_ps.tile([P, 512], F32, tag="pjps")
                    for et in range(ET):
                        nc.tensor.matmul(
                            ps,
                            lhsT=x_sb[:, et, nb * P:(nb + 1) * P],
                            rhs=wv_sb[:, et, :],
                            start=(et == 0), stop=(et == ET - 1),
                        )
                    evict(v_all[:, nch * 4 + nb, :, :D],
                          ps.rearrange("p (h d) -> p h d", h=HPC), nch == 0)
                return run

            for w_sb, dst in ((wq_sb, qs), (wk_sb, ks)):
                for t in range(HPC):
                    groups.append(qk_group(w_sb, dst, t))
            for nb in range(4):
                groups.append(v_group(nb))
            return groups

        # ---------------- attention micro-op closures ----------------
        stage_pt = {}

        def qk_ops(h, ci):
            """One op per QK^T pair: 2 matmuls + exp; masks on the last."""
            BJ = 4 * (ci + 1)
            ops = []

            def first_extra(pt):
                # clear regions partial-exp never writes on this rotating
                # buffer (exactly the mask-0 prefix of the last diag pair)
                nc.gpsimd.memset(pt[:, BJ - 2, :256], 0.0)
                nc.gpsimd.memset(pt[:, BJ - 1, :384], 0.0)

            def pair_op(bjp):
                def run():
                    if bjp == 0:
                        pt = pt_pool.tile([P, ET, 512], FP16, tag="pt",
                                          name=f"pt{h}_{ci}")
                        stage_pt[(h, ci)] = pt
                        first_extra(pt)
                    pt = stage_pt[(h, ci)]
                    ps = qk_ps.tile([P, 2, 512], F32, tag="qkps")
                    last_pair = (bjp == BJ // 2 - 1)
                    for u in range(2):
                        bj = 2 * bjp + u
                        rr = bj - (BJ - 4)
                        if rr > 0:
                            nc.tensor.matmul(
                                ps[:, u, rr * P:],
                                lhsT=ks[:, h, bj * P:(bj + 1) * P],
                                rhs=qs[:, h,
                                       ci * 512 + rr * P:(ci + 1) * 512],
                                start=True, stop=True,
                            )
                        else:
                            nc.tensor.matmul(
                                ps[:, u, :],
                                lhsT=ks[:, h, bj * P:(bj + 1) * P],
                                rhs=qs[:, h, ci * 512:(ci + 1) * 512],
                                start=True, stop=True,
                            )
                    if last_pair:
                        # diagonal blocks r=256,384: only cols >= r valid
                        nc.scalar.activation(
                            out=pt[:, 2 * bjp, 256:], in_=ps[:, 0, 256:],
                            func=mybir.ActivationFunctionType.Exp,
                        )
                        nc.scalar.activation(
                            out=pt[:, 2 * bjp + 1, 384:], in_=ps[:, 1, 384:],
                            func=mybir.ActivationFunctionType.Exp,
                        )
                        # causal masks on the diagonal blocks; full-tile so
                        # the zero prefix also clears stale/garbage regions
                        for rr2 in range(4):
                            bj2 = BJ - 4 + rr2
                            nc.vector.tensor_mul(
                                out=pt[:, bj2, :], in0=pt[:, bj2, :],
                                in1=mask_sb[:, rr2, :])
                    else:
                        nc.scalar.activation(
                            out=pt[:, 2 * bjp:2 * bjp + 2, :], in_=ps,
                            func=mybir.ActivationFunctionType.Exp,
                        )
                return run

            for bjp in range(BJ // 2):
                ops.append(pair_op(bjp))
            return ops

        def av_ops(h, ci):
            """One op per A@[V|1] psum; spill+transpose on the last."""
            ops = []

            def ib_op(ib):
                def run():
                    pt = stage_pt[(h, ci)]
                    if ib == 0:
                        att = att_pool.tile([P, 4, D], FP16, tag="atth",
                                            name=f"att{h}_{ci}")
                        stage_pt[(h, ci, "att")] = att
                    att = stage_pt[(h, ci, "att")]
                    nbj = 4 * ci + ib + 1
                    avp = av_ps.tile([P, D + 4], F32, tag="avps")
                    isl = slice(ib * P, (ib + 1) * P)
                    for bj in range(nbj):
                        nc.tensor.matmul(
                            avp[:, :D + 1],
                            lhsT=pt[:, bj, isl],
                            rhs=v_all[:, bj, h, :D + 1],
                            start=(bj == 0), stop=(bj == nbj - 1),
                        )
                    rs = rs_pool.tile([P, 1], F32, tag="rs")
                    nc.vector.reciprocal_approx_fast(
                        out=rs, in_=avp[:, D:D + 1])
                    nc.vector.tensor_scalar_mul(
                        out=att[:, ib, :], in0=avp[:, :D], scalar1=rs)
                    if ib == 3:
                        # spill + transpose right away so the out-projection
                        # isn't gated on the whole head
                        nc.sync.dma_start(
                            out=attd[h, ci * 512:(ci + 1) * 512, :].rearrange(
                                "(io p) d -> p io d", p=P),
                            in_=att)
                        nc.sync.dma_start_transpose(
                            out=outTs[h][:, ci * 512:(ci + 1) * 512],
                            in_=attd[h, ci * 512:(ci + 1) * 512, :])
                return run

            for ib in range(4):
                ops.append(ib_op(ib))
            return ops

        def stage_ops(ci, pending):
            """Micro-ops for all 4 heads of chunk ci; A@V lags one head so
            exp always has at least a head's worth of PE runway. Returns
            (ops, new_pending) with the last head's A@V deferred."""
            ops = []
            for h in range(HPC):
                ops += qk_ops(h, ci)
                if h == 0:
                    ops += pending
                else:
                    ops += av_ops(h - 1, ci)
            return ops, av_ops(HPC - 1, ci)

        # ---------------- interleaved proj + attention ----------------
        pending = []
        micro = []
        for nch in range(NCH):
            if nch >= 1:
                micro, pending = stage_ops(nch - 1, pending)
            groups = proj_groups(nch)
            ng = len(groups)
            done = 0
            for i, g in enumerate(groups):
                g()
                target = (i + 1) * len(micro) // ng
                while done < target:
                    micro[done]()
                    done += 1
            if nch + 2 < NCH:
                load_x(nch + 2)

        # ---------------- tail: last attention chunk + out-projection -----
        tail_ops, pending = stage_ops(NCH - 1, pending)

        # proj inputs are dead now; reuse their SBUF for the oproj weights
        pj_ps.release()
        xpool.release()
        wpool.release()
        wo_pool = tc.alloc_tile_pool(name="wo_pool", bufs=1)
        wo_sb = wo_pool.tile([P, HPC, E], FP16)
        for ec in range(NCH):
            esl = slice(ec * 512, (ec + 1) * 512)
            nc.sync.dma_start(out=wo_sb[:, :, esl], in_=woT_r[:, :, esl])

        op_ps = tc.alloc_tile_pool(name="op_ps", bufs=2, space="PSUM")
        op_ev = tc.alloc_tile_pool(name="op_ev", bufs=3)
        _oev = [0]

        def oproj_group(nb, ec, ostage):
            ps = op_ps.tile([P, 512], F32, tag="opps")
            for t in range(HPC):
                nc.tensor.matmul(
                    ps,
                    lhsT=outTs[t][:, nb * P:(nb + 1) * P],
                    rhs=wo_sb[:, t, ec * 512:(ec + 1) * 512],
                    start=(t == 0), stop=(t == HPC - 1),
                )
            _copies[_oev[0] % 2](out=ostage[:, ec, :], in_=ps)
            _oev[0] += 1

        _dmas2 = (nc.sync, nc.scalar)

        def oproj_nb(nb, split_dma=False):
            ostage = op_ev.tile([P, NCH, 512], FP16, tag="opev",
                                name=f"ostage{nb}")
            for ec in range(NCH):
                eng = _oev[0] % 2
                oproj_group(nb, ec, ostage)
                if split_dma:
                    # final blocks: stream per-ec pieces from the queue of
                    # the engine that evicted them, so the kernel tail is
                    # several concurrent small DMAs, not one 512KB one
                    _dmas2[eng].dma_start(
                        out=out.ap()[nb * P:(nb + 1) * P,
                                     ec * 512:(ec + 1) * 512],
                        in_=ostage[:, ec, :])
            if not split_dma:
                nc.sync.dma_start(
                    out=out.ap()[nb * P:(nb + 1) * P, :], in_=ostage)

        # pace the last attention chunk across oproj blocks 0..7 (only
        # chunks ci<=2 feed them, all transposed already) at ec-group
        # granularity so qk pairs never burst past the 2-deep PSUM window;
        # run the first few attention ops up front to hide the wo_sb DMA
        done = 0
        while done < len(tail_ops) // 9:
            tail_ops[done]()
            done += 1
        n_ec = 8 * NCH
        ostages = {}
        for i in range(n_ec):
            nb, ec = divmod(i, NCH)
            if ec == 0:
                ostages[nb] = op_ev.tile([P, NCH, 512], FP16, tag="opev",
                                         name=f"ostage{nb}")
            oproj_group(nb, ec, ostages[nb])
            if ec == NCH - 1:
                nc.sync.dma_start(
                    out=out.ap()[nb * P:(nb + 1) * P, :], in_=ostages[nb])
            target = len(tail_ops) // 9 + (i + 1) * (
                len(tail_ops) - len(tail_ops) // 9) // n_ec
            while done < target:
                tail_ops[done]()
                done += 1
        while done < len(tail_ops):
            tail_ops[done]()
            done += 1
        oproj_nb(8)
        oproj_nb(9)
        for op in pending:   # A@V of the very last stage (exp has had runway)
            op()
        for nb in range(10, NB):
            oproj_nb(nb, split_dma=(nb >= NB - 2))

        for pool in (op_ev, op_ps, wo_pool, av_ps, qk_ps, rs_pool, att_pool,
                     pt_pool, outT_pool, big, dram, consts):
            pool.release()

    nc.compile()
    return nc


def make_in_maps(x, Wq, Wkv, Wout):
    x = np.asarray(x, dtype=np.float32)
    Wq = np.asarray(Wq, dtype=np.float32)
    Wkv = np.asarray(Wkv, dtype=np.float32)
    Wout = np.asarray(Wout, dtype=np.float32)
    scale = np.float32(D ** -0.5)

    # causal masks for the 4 diagonal offsets
    jj = np.arange(P)[:, None]
    ii = np.arange(512)[None, :]
    mask = np.zeros((P, 4, 512), dtype=np.float16)
    for rr in range(4):
        mask[:, rr, :] = (ii >= jj + rr * P).astype(np.float16)

    xT = [np.ascontiguousarray(x[b].T).astype(np.float16) for b in range(B)]
    in_maps = []
    for c in range(NCORES):
        b, hg = divmod(c, 4)
        sl = slice(hg * DC, (hg + 1) * DC)
        in_maps.append({
            "xT": xT[b],
            "wqT": (np.ascontiguousarray(Wq[sl, :].T) * scale).astype(np.float16),
            "wkT": np.ascontiguousarray(Wkv[sl, :].T).astype(np.float16),
            "wvT": np.ascontiguousarray(Wkv[E + sl.start:E + sl.stop, :].T).astype(np.float16),
            "woT": np.ascontiguousarray(Wout[:, sl].T).astype(np.float16),
            "maskin": mask,
        })
    return in_maps


_NC_CACHE = []


def _get_nc():
    if not _NC_CACHE:
        _NC_CACHE.append(build_nc())
    return _NC_CACHE[0]


def _run(in_maps):
    nc = _get_nc()
    return run_bass_kernel_spmd(nc, in_maps, core_ids=list(range(NCORES)))


def kernel(x, Wq, Wkv, Wout):
    in_maps = make_in_maps(x, Wq, Wkv, Wout)
    res = _run(in_maps)
    out = np.zeros((B, N, E), dtype=np.float32)
    for c in range(NCORES):
        out[c // 4] += res.results[c]["out"].astype(np.float32)
    return out


if __name__ == "__main__":
    t0 = time.time()
    _get_nc()
    print(f"build+compile: {time.time() - t0:.1f}s")


# revision 27
# speedup vs baseline: 1.0842x; 1.0163x over previous
"""Trainium2 Bass kernel for nn_BaseAttention (B=2, N=2048, E=2048, H=16, D=128).

Sharding: 8 cores; core c handles batch b=c//4, head-group hg=c%4 (4 heads).
Each core computes q/k/v projections for its heads, causal flash-style
attention, and a partial out-projection (contraction over its 512 head dims).
Host sums the 4 partial outputs per batch (tensor-parallel unshard).

All matmuls fp16 (1 row/cycle at 2.4 GHz). Causal structure is exploited at
128-row granularity: diagonal QK^T blocks stream only causally-valid rows and
A@V skips fully-masked j-blocks. Attention for n-chunk c is interleaved into
projection chunk c+1 as micro-ops paced between PSUM groups, so ScalarE's exp
(the second-largest engine load) hides under projection matmuls instead of
gating the PE; the attention tail similarly interleaves with the
out-projection. PSUM evictions round-robin over DVE/Pool, keeping ACT free
for exp. Output partials are emitted as fp16 (summed in f32 on host).
"""

import os
import sys
import time

sys.path.insert(0, "/opt/trn_rl_repo")

import numpy as np
import ml_dtypes

import concourse.bass as bass
import concourse.mybir as mybir
import concourse.tile as tile
from concourse import bacc
from concourse.bass_utils import run_bass_kernel_spmd

B, N, E, H = 2, 2048, 2048, 16
D = E // H            # 128
HPC = 4               # heads per core
DC = HPC * D          # 512 head dims per core
NCORES = 8
P = 128
NCH = N // 512        # 4 n-chunks of 512
ET = E // P           # 16 e-tiles of 128
NB = N // P           # 16 n-blocks of 128

F32 = mybir.dt.float32
FP16 = mybir.dt.float16


def build_nc():
    nc = bacc.Bacc("TRN2", target_bir_lowering=False, debug=False,
                   num_devices=NCORES)

    xT = nc.dram_tensor("xT", [E, N], FP16, kind="ExternalInput")
    wqT = nc.dram_tensor("wqT", [E, DC], FP16, kind="ExternalInput")
    wkT = nc.dram_tensor("wkT", [E, DC], FP16, kind="ExternalInput")
    wvT = nc.dram_tensor("wvT", [E, DC], FP16, kind="ExternalInput")
    woT = nc.dram_tensor("woT", [DC, E], FP16, kind="ExternalInput")
    maskin = nc.dram_tensor("maskin", [P, 4, 512], FP16, kind="ExternalInput")
    out = nc.dram_tensor("out", [N, E], FP16, kind="ExternalOutput")

    xT_r = xT.ap().rearrange("(eo p) n -> p eo n", p=P)      # [128,16,2048]
    wqT_r = wqT.ap().rearrange("(eo p) d -> p eo d", p=P)    # [128,16,512]
    wkT_r = wkT.ap().rearrange("(eo p) d -> p eo d", p=P)
    wvT_r = wvT.ap().rearrange("(eo p) d -> p eo d", p=P)
    woT_r = woT.ap().rearrange("(t p) e -> p t e", p=P)      # [128,4,2048]

    with tile.TileContext(nc) as tc:
        # ---------------- long-lived pools ----------------
        consts = tc.alloc_tile_pool(name="consts", bufs=1)
        mask_sb = consts.tile([P, 4, 512], FP16)
        # prefire the Exp table load so it overlaps the input DMA head
        dummy = consts.tile([1, 8], F32)
        nc.vector.memset(dummy, 0.0)
        nc.scalar.activation(out=dummy, in_=dummy,
                             func=mybir.ActivationFunctionType.Exp)

        dram = tc.alloc_tile_pool(name="dram", bufs=1, space="DRAM")
        attd = dram.tile([HPC, N, D], FP16)          # normalized attn out

        big = tc.alloc_tile_pool(name="big", bufs=1)
        qs = big.tile([P, HPC, N], FP16)                  # q^T, heads stacked
        ks = big.tile([P, HPC, N], FP16)                  # k^T
        v_all = big.tile([P, NB, HPC, D + 4], FP16)       # [V | 1] per block
        nc.vector.memset(v_all[:, :, :, D:D + 1], 1.0)

        outT_pool = tc.alloc_tile_pool(name="outT", bufs=1)
        outTs = [outT_pool.tile([P, N], FP16, name=f"outT{t}")
                 for t in range(HPC)]

        # ---------------- attention pools ----------------
        pt_pool = tc.alloc_tile_pool(name="pt_pool", bufs=3)
        att_pool = tc.alloc_tile_pool(name="att_pool", bufs=2)
        rs_pool = tc.alloc_tile_pool(name="rs_pool", bufs=8)
        qk_ps = tc.alloc_tile_pool(name="qk_ps", bufs=2, space="PSUM")
        av_ps = tc.alloc_tile_pool(name="av_ps", bufs=2, space="PSUM")
        # ---- proj-phase pools, on top of the stack (released before oproj)
        wpool = tc.alloc_tile_pool(name="wpool", bufs=1)
        xpool = tc.alloc_tile_pool(name="xpool", bufs=2)
        pj_ps = tc.alloc_tile_pool(name="pj_ps", bufs=2, space="PSUM")

        wq_sb = wpool.tile([P, ET, DC], FP16)
        wk_sb = wpool.tile([P, ET, DC], FP16)
        wv_sb = wpool.tile([P, ET, DC], FP16)
        x_tiles = [None] * NCH

        def load_x(nch):
            # on the SP queue: the Pool queue carries evictions, and this
            # DMA's WAR wait on the old buffer would head-of-line block them
            t = xpool.tile([P, ET, 512], FP16, tag="xchunk",
                           name=f"x_sb{nch}")
            nc.sync.dma_start(
                out=t, in_=xT_r[:, :, nch * 512:(nch + 1) * 512])
            x_tiles[nch] = t

        # two DMA queues (SP + Pool) with wq/x0 split in eo-pairs and issued
        # alternately: DMA transfers serialize globally in trigger order, so
        # arrival must match consumption: wq/x0 interleaved (first qk
        # groups), then wk, wv, mask, and x1 last (needed only at chunk 1)
        x0 = xpool.tile([P, ET, 512], FP16, tag="xchunk", name="x_sb0")
        x_tiles[0] = x0
        nc.sync.dma_start(out=wq_sb[:, 0:1, :], in_=wqT_r[:, 0:1, :])
        nc.gpsimd.dma_start(out=x0[:, 0:1, :], in_=xT_r[:, 0:1, 0:512])
        nc.sync.dma_start(out=wq_sb[:, 1:2, :], in_=wqT_r[:, 1:2, :])
        nc.gpsimd.dma_start(out=x0[:, 1:2, :], in_=xT_r[:, 1:2, 0:512])
        for g in range(1, ET // 2):
            gs = slice(2 * g, 2 * g + 2)
            nc.sync.dma_start(out=wq_sb[:, gs, :], in_=wqT_r[:, gs, :])
            nc.gpsimd.dma_start(out=x0[:, gs, :], in_=xT_r[:, gs, 0:512])
        for g in range(ET // 2):
            gs = slice(2 * g, 2 * g + 2)
            nc.sync.dma_start(out=wk_sb[:, gs, :], in_=wkT_r[:, gs, :])
        QT = ET // 4
        for g in range(4):
            gs = slice(g * QT, (g + 1) * QT)
            nc.gpsimd.dma_start(out=wv_sb[:, gs, :], in_=wvT_r[:, gs, :])
        nc.gpsimd.dma_start(out=mask_sb, in_=maskin.ap())
        x1 = xpool.tile([P, ET, 512], FP16, tag="xchunk", name="x_sb1")
        x_tiles[1] = x1
        nc.gpsimd.dma_start(out=x1[:, :ET // 2, :],
                            in_=xT_r[:, :ET // 2, 512:1024])
        nc.gpsimd.dma_start(out=x1[:, ET // 2:, :],
                            in_=xT_r[:, ET // 2:, 512:1024])

        # PSUM evictions: only DVE and ACT can read PSUM (GPSIMD cannot).
        # ACT is reserved for exp while attention is interleaved, so it only
        # helps during chunk 0 and the out-projection tail.
        _ev = [0]
        _copies = (nc.vector.tensor_copy, nc.scalar.copy)

        def evict(out_ap, in_ap, use_act):
            n = 2 if use_act else 1
            _copies[_ev[0] % n](out=out_ap, in_=in_ap)
            _ev[0] += 1

        # ---------------- proj psum-group closures ----------------
        def proj_groups(nch):
            x_sb = x_tiles[nch]
            nsl = slice(nch * 512, (nch + 1) * 512)
            groups = []

            def qk_group(w_sb, dst, t):
                def run():
                    ps = pj_ps.tile([P, 512], F32, tag="pjps")
                    for et in range(ET):
                        nc.tensor.matmul(
                            ps,
                            lhsT=w_sb[:, et, t * P:(t + 1) * P],
                            rhs=x_sb[:, et, :],
                            start=(et == 0), stop=(et == ET - 1),
                        )
                    evict(dst[:, t, nsl], ps, nch == 0)
                return run

            def v_group(nb):
                def run():
                    ps = pj_ps.tile([P, 512], F32, tag="pjps")
                    for et in range(ET):
                        nc.tensor.matmul(
                            ps,
                            lhsT=x_sb[:, et, nb * P:(nb + 1) * P],
                            rhs=wv_sb[:, et, :],
                            start=(et == 0), stop=(et == ET - 1),
                        )
                    evict(v_all[:, nch * 4 + nb, :, :D],
                          ps.rearrange("p (h d) -> p h d", h=HPC), nch == 0)
                return run

            for w_sb, dst in ((wq_sb, qs), (wk_sb, ks)):
                for t in range(HPC):
                    groups.append(qk_group(w_sb, dst, t))
            for nb in range(4):
                groups.append(v_group(nb))
            return groups

        def proj_groups_c0():
            """Chunk 0 is DMA-arrival-bound: emit quarter-contraction steps
            round-robining all four heads, borrowing the (idle) attention
            qk_ps pair tiles so four psums are in flight and the PE always
            has work per arriving wq/x0 eo-pair."""
            x_sb = x_tiles[0]
            nsl = slice(0, 512)
            state = {}
            groups = []

            def qk_quarter(w_sb, dst, key, t, u, q):
                def run():
                    if q == 0 and u == 0:
                        state[key] = qk_ps.tile([P, 2, 512], F32, tag="qkps",
                                                name=f"c0{key}")
                    ps = state[key]
                    for et in range(4 * q, 4 * q + 4):
                        nc.tensor.matmul(
                            ps[:, u, :],
                            lhsT=w_sb[:, et, t * P:(t + 1) * P],
                            rhs=x_sb[:, et, :],
                            start=(et == 0), stop=(et == ET - 1),
                        )
                    if q == 3 and u == 1:
                        evict(dst[:, t - 1:t + 1, nsl], ps, True)
                return run

            def v_quarter(key, nbp, u, q):
                def run():
                    if q == 0 and u == 0:
                        state[key] = qk_ps.tile([P, 2, 512], F32, tag="qkps",
                                                name=f"c0{key}")
                    ps = state[key]
                    nb = 2 * nbp + u
                    for et in range(4 * q, 4 * q + 4):
                        nc.tensor.matmul(
                            ps[:, u, :],
                            lhsT=x_sb[:, et, nb * P:(nb + 1) * P],
                            rhs=wv_sb[:, et, :],
                            start=(et == 0), stop=(et == ET - 1),
                        )
                    if q == 3 and u == 1:
                        evict(v_all[:, 2 * nbp:2 * nbp + 2, :, :D],
                              ps.rearrange("p g (h d) -> p g h d", h=HPC),
                              True)
                return run

            for w_sb, dst, wn in ((wq_sb, qs, "q"), (wk_sb, ks, "k")):
                for q in range(4):
                    for pi in range(2):
                        for u in range(2):
                            groups.append(
                                qk_quarter(w_sb, dst, wn + str(pi),
                                           2 * pi + u, u, q))
            for q in range(4):
                for nbp in range(2):
                    for u in range(2):
                        groups.append(v_quarter("v" + str(nbp), nbp, u, q))
            return groups

        # ---------------- attention micro-op closures ----------------
        stage_pt = {}

        def qk_ops(h, ci):
            """One op per QK^T pair: 2 matmuls + exp; masks on the last."""
            BJ = 4 * (ci + 1)
            ops = []

            def first_extra(pt):
                # clear regions partial-exp never writes on this rotating
                # buffer (exactly the mask-0 prefix of the last diag pair)
                nc.gpsimd.memset(pt[:, BJ - 2, :256], 0.0)
                nc.gpsimd.memset(pt[:, BJ - 1, :384], 0.0)

            def pair_op(bjp):
                def run():
                    if bjp == 0:
                        pt = pt_pool.tile([P, ET, 512], FP16, tag="pt",
                                          name=f"pt{h}_{ci}")
                        stage_pt[(h, ci)] = pt
                        first_extra(pt)
                    pt = stage_pt[(h, ci)]
                    ps = qk_ps.tile([P, 2, 512], F32, tag="qkps")
                    last_pair = (bjp == BJ // 2 - 1)
                    for u in range(2):
                        bj = 2 * bjp + u
                        rr = bj - (BJ - 4)
                        if rr > 0:
                            nc.tensor.matmul(
                                ps[:, u, rr * P:],
                                lhsT=ks[:, h, bj * P:(bj + 1) * P],
                                rhs=qs[:, h,
                                       ci * 512 + rr * P:(ci + 1) * 512],
                                start=True, stop=True,
                            )
                        else:
                            nc.tensor.matmul(
                                ps[:, u, :],
                                lhsT=ks[:, h, bj * P:(bj + 1) * P],
                                rhs=qs[:, h, ci * 512:(ci + 1) * 512],
                                start=True, stop=True,
                            )
                    if last_pair:
                        # diagonal blocks r=256,384: only cols >= r valid
                        nc.scalar.activation(
                            out=pt[:, 2 * bjp, 256:], in_=ps[:, 0, 256:],
                            func=mybir.ActivationFunctionType.Exp,
                        )
                        nc.scalar.activation(
                            out=pt[:, 2 * bjp + 1, 384:], in_=ps[:, 1, 384:],
                            func=mybir.ActivationFunctionType.Exp,
                        )
                        # causal masks on the diagonal blocks; full-tile so
                        # the zero prefix also clears stale/garbage regions
                        for rr2 in range(4):
                            bj2 = BJ - 4 + rr2
                            nc.vector.tensor_mul(
                                out=pt[:, bj2, :], in0=pt[:, bj2, :],
                                in1=mask_sb[:, rr2, :])
                    else:
                        nc.scalar.activation(
                            out=pt[:, 2 * bjp:2 * bjp + 2, :], in_=ps,
                            func=mybir.ActivationFunctionType.Exp,
                        )
                return run

            for bjp in range(BJ // 2):
                ops.append(pair_op(bjp))
            return ops

        def av_ops(h, ci):
            """One op per A@[V|1] psum; spill+transpose on the last."""
            ops = []

            def ib_op(ib):
                def run():
                    pt = stage_pt[(h, ci)]
                    if ib == 0:
                        att = att_pool.tile([P, 4, D], FP16, tag="atth",
                                            name=f"att{h}_{ci}")
                        stage_pt[(h, ci, "att")] = att
                    att = stage_pt[(h, ci, "att")]
                    nbj = 4 * ci + ib + 1
                    avp = av_ps.tile([P, D + 4], F32, tag="avps")
                    isl = slice(ib * P, (ib + 1) * P)
                    for bj in range(nbj):
                        nc.tensor.matmul(
                            avp[:, :D + 1],
                            lhsT=pt[:, bj, isl],
                            rhs=v_all[:, bj, h, :D + 1],
                            start=(bj == 0), stop=(bj == nbj - 1),
                        )
                    rs = rs_pool.tile([P, 1], F32, tag="rs")
                    nc.vector.reciprocal_approx_fast(
                        out=rs, in_=avp[:, D:D + 1])
                    nc.vector.tensor_scalar_mul(
                        out=att[:, ib, :], in0=avp[:, :D], scalar1=rs)
                    if ib == 3:
                        # spill + transpose right away so the out-projection
                        # isn't gated on the whole head
                        nc.sync.dma_start(
                            out=attd[h, ci * 512:(ci + 1) * 512, :].rearrange(
                                "(io p) d -> p io d", p=P),
                            in_=att)
                        nc.sync.dma_start_transpose(
                            out=outTs[h][:, ci * 512:(ci + 1) * 512],
                            in_=attd[h, ci * 512:(ci + 1) * 512, :])
                return run

            for ib in range(4):
                ops.append(ib_op(ib))
            return ops

        def stage_ops(ci, pending):
            """Micro-ops for all 4 heads of chunk ci; A@V lags one head so
            exp always has at least a head's worth of PE runway. Returns
            (ops, new_pending) with the last head's A@V deferred."""
            ops = []
            for h in range(HPC):
                ops += qk_ops(h, ci)
                if h == 0:
                    ops += pending
                else:
                    ops += av_ops(h - 1, ci)
            return ops, av_ops(HPC - 1, ci)

        # ---------------- interleaved proj + attention ----------------
        pending = []
        micro = []
        for nch in range(NCH):
            if nch >= 1:
                micro, pending = stage_ops(nch - 1, pending)
            groups = proj_groups(nch) if nch else proj_groups_c0()
            ng = len(groups)
            done = 0
            for i, g in enumerate(groups):
                g()
                target = (i + 1) * len(micro) // ng
                while done < target:
                    micro[done]()
                    done += 1
            if nch + 2 < NCH:
                load_x(nch + 2)

        # ---------------- tail: last attention chunk + out-projection -----
        tail_ops, pending = stage_ops(NCH - 1, pending)

        # proj inputs are dead now; reuse their SBUF for the oproj weights
        pj_ps.release()
        xpool.release()
        wpool.release()
        wo_pool = tc.alloc_tile_pool(name="wo_pool", bufs=1)
        wo_sb = wo_pool.tile([P, HPC, E], FP16)
        for ec in range(NCH):
            esl = slice(ec * 512, (ec + 1) * 512)
            nc.sync.dma_start(out=wo_sb[:, :, esl], in_=woT_r[:, :, esl])

        op_ps = tc.alloc_tile_pool(name="op_ps", bufs=2, space="PSUM")
        op_ev = tc.alloc_tile_pool(name="op_ev", bufs=3)
        _oev = [0]

        def oproj_group(nb, ec, ostage):
            ps = op_ps.tile([P, 512], F32, tag="opps")
            for t in range(HPC):
                nc.tensor.matmul(
                    ps,
                    lhsT=outTs[t][:, nb * P:(nb + 1) * P],
                    rhs=wo_sb[:, t, ec * 512:(ec + 1) * 512],
                    start=(t == 0), stop=(t == HPC - 1),
                )
            _copies[_oev[0] % 2](out=ostage[:, ec, :], in_=ps)
            _oev[0] += 1

        _dmas2 = (nc.sync, nc.scalar)

        def oproj_final_ec(nb, ec, ostage):
            # very last psum group: halve everything so the critical chain
            # after the final matmul is one small evict + two parallel DMAs
            ps = op_ps.tile([P, 512], F32, tag="opps", name="opps_fin")
            orow = out.ap()[nb * P:(nb + 1) * P, :]
            for half in range(2):
                csl = slice(half * 256, (half + 1) * 256)
                for t in range(HPC):
                    nc.tensor.matmul(
                        ps[:, csl],
                        lhsT=outTs[t][:, nb * P:(nb + 1) * P],
                        rhs=wo_sb[:, t, ec * 512 + half * 256:
                                  ec * 512 + (half + 1) * 256],
                        start=(t == 0), stop=(t == HPC - 1),
                    )
                _copies[half](out=ostage[:, ec, csl], in_=ps[:, csl])
                _dmas2[half].dma_start(
                    out=orow[:, ec * 512 + half * 256:
                             ec * 512 + (half + 1) * 256],
                    in_=ostage[:, ec, csl])

        def oproj_nb(nb, split_dma=False):
            ostage = op_ev.tile([P, NCH, 512], FP16, tag="opev",
                                name=f"ostage{nb}")
            for ec in range(NCH):
                eng = _oev[0] % 2
                if split_dma and nb == NB - 1 and ec == NCH - 1:
                    oproj_final_ec(nb, ec, ostage)
                    continue
                oproj_group(nb, ec, ostage)
                if split_dma:
                    # final blocks: stream per-ec pieces from the queue of
                    # the engine that evicted them, so the kernel tail is
                    # several concurrent small DMAs, not one 512KB one
                    _dmas2[eng].dma_start(
                        out=out.ap()[nb * P:(nb + 1) * P,
                                     ec * 512:(ec + 1) * 512],
                        in_=ostage[:, ec, :])
            if not split_dma:
                nc.sync.dma_start(
                    out=out.ap()[nb * P:(nb + 1) * P, :], in_=ostage)

        # pace the last attention chunk across oproj blocks 0..7 (only
        # chunks ci<=2 feed them, all transposed already) at ec-group
        # granularity so qk pairs never burst past the 2-deep PSUM window;
        # run the first few attention ops up front to hide the wo_sb DMA
        done = 0
        while done < len(tail_ops) // 9:
            tail_ops[done]()
            done += 1
        n_ec = 8 * NCH
        ostages = {}
        for i in range(n_ec):
            nb, ec = divmod(i, NCH)
            if ec == 0:
                ostages[nb] = op_ev.tile([P, NCH, 512], FP16, tag="opev",
                                         name=f"ostage{nb}")
            oproj_group(nb, ec, ostages[nb])
            if ec == NCH - 1:
                nc.sync.dma_start(
                    out=out.ap()[nb * P:(nb + 1) * P, :], in_=ostages[nb])
            target = len(tail_ops) // 9 + (i + 1) * (
                len(tail_ops) - len(tail_ops) // 9) // n_ec
            while done < target:
                tail_ops[done]()
                done += 1
        while done < len(tail_ops):
            tail_ops[done]()
            done += 1
        oproj_nb(8)
        oproj_nb(9)
        for op in pending:   # A@V of the very last stage (exp has had runway)
            op()
        for nb in range(10, NB):
            oproj_nb(nb, split_dma=(nb >= NB - 2))

        for pool in (op_ev, op_ps, wo_pool, av_ps, qk_ps, rs_pool, att_pool,
                     pt_pool, outT_pool, big, dram, consts):
            pool.release()

    nc.compile()
    return nc


def make_in_maps(x, Wq, Wkv, Wout):
    x = np.asarray(x, dtype=np.float32)
    Wq = np.asarray(Wq, dtype=np.float32)
    Wkv = np.asarray(Wkv, dtype=np.float32)
    Wout = np.asarray(Wout, dtype=np.float32)
    scale = np.float32(D ** -0.5)

    # causal masks for the 4 diagonal offsets
    jj = np.arange(P)[:, None]
    ii = np.arange(512)[None, :]
    mask = np.zeros((P, 4, 512), dtype=np.float16)
    for rr in range(4):
        mask[:, rr, :] = (ii >= jj + rr * P).astype(np.float16)

    xT = [np.ascontiguousarray(x[b].T).astype(np.float16) for b in range(B)]
    in_maps = []
    for c in range(NCORES):
        b, hg = divmod(c, 4)
        sl = slice(hg * DC, (hg + 1) * DC)
        in_maps.append({
            "xT": xT[b],
            "wqT": (np.ascontiguousarray(Wq[sl, :].T) * scale).astype(np.float16),
            "wkT": np.ascontiguousarray(Wkv[sl, :].T).astype(np.float16),
            "wvT": np.ascontiguousarray(Wkv[E + sl.start:E + sl.stop, :].T).astype(np.float16),
            "woT": np.ascontiguousarray(Wout[:, sl].T).astype(np.float16),
            "maskin": mask,
        })
    return in_maps


_NC_CACHE = []


def _get_nc():
    if not _NC_CACHE:
        _NC_CACHE.append(build_nc())
    return _NC_CACHE[0]


def _run(in_maps):
    nc = _get_nc()
    return run_bass_kernel_spmd(nc, in_maps, core_ids=list(range(NCORES)))


def kernel(x, Wq, Wkv, Wout):
    in_maps = make_in_maps(x, Wq, Wkv, Wout)
    res = _run(in_maps)
    out = np.zeros((B, N, E), dtype=np.float32)
    for c in range(NCORES):
        out[c // 4] += res.results[c]["out"].astype(np.float32)
    return out


if __name__ == "__main__":
    t0 = time.time()
    _get_nc()
    print(f"build+compile: {time.time() - t0:.1f}s")


# revision 30
# speedup vs baseline: 1.0874x; 1.0030x over previous
"""Trainium2 Bass kernel for nn_BaseAttention (B=2, N=2048, E=2048, H=16, D=128).

Sharding: 8 cores; core c handles batch b=c//4, head-group hg=c%4 (4 heads).
Each core computes q/k/v projections for its heads, causal flash-style
attention, and a partial out-projection (contraction over its 512 head dims).
Host sums the 4 partial outputs per batch (tensor-parallel unshard).

All matmuls fp16 (1 row/cycle at 2.4 GHz). Causal structure is exploited at
128-row granularity: diagonal QK^T blocks stream only causally-valid rows and
A@V skips fully-masked j-blocks. Attention for n-chunk c is interleaved into
projection chunk c+1 as micro-ops paced between PSUM groups, so ScalarE's exp
(the second-largest engine load) hides under projection matmuls instead of
gating the PE; the attention tail similarly interleaves with the
out-projection. PSUM evictions round-robin over DVE/Pool, keeping ACT free
for exp. Output partials are emitted as fp16 (summed in f32 on host).
"""

import os
import sys
import time

sys.path.insert(0, "/opt/trn_rl_repo")

import numpy as np
import ml_dtypes

import concourse.bass as bass
import concourse.mybir as mybir
import concourse.tile as tile
from concourse import bacc
from concourse.bass_utils import run_bass_kernel_spmd

B, N, E, H = 2, 2048, 2048, 16
D = E // H            # 128
HPC = 4               # heads per core
DC = HPC * D          # 512 head dims per core
NCORES = 8
P = 128
NCH = N // 512        # 4 n-chunks of 512
ET = E // P           # 16 e-tiles of 128
NB = N // P           # 16 n-blocks of 128

F32 = mybir.dt.float32
FP16 = mybir.dt.float16


def build_nc():
    nc = bacc.Bacc("TRN2", target_bir_lowering=False, debug=False,
                   num_devices=NCORES)

    xT = nc.dram_tensor("xT", [E, N], FP16, kind="ExternalInput")
    wqT = nc.dram_tensor("wqT", [E, DC], FP16, kind="ExternalInput")
    wkT = nc.dram_tensor("wkT", [E, DC], FP16, kind="ExternalInput")
    wvT = nc.dram_tensor("wvT", [E, DC], FP16, kind="ExternalInput")
    woT = nc.dram_tensor("woT", [DC, E], FP16, kind="ExternalInput")
    maskin = nc.dram_tensor("maskin", [P, 4, 512], FP16, kind="ExternalInput")
    out = nc.dram_tensor("out", [N, E], FP16, kind="ExternalOutput")

    xT_r = xT.ap().rearrange("(eo p) n -> p eo n", p=P)      # [128,16,2048]
    wqT_r = wqT.ap().rearrange("(eo p) d -> p eo d", p=P)    # [128,16,512]
    wkT_r = wkT.ap().rearrange("(eo p) d -> p eo d", p=P)
    wvT_r = wvT.ap().rearrange("(eo p) d -> p eo d", p=P)
    woT_r = woT.ap().rearrange("(t p) e -> p t e", p=P)      # [128,4,2048]

    with tile.TileContext(nc) as tc:
        # ---------------- long-lived pools ----------------
        consts = tc.alloc_tile_pool(name="consts", bufs=1)
        mask_sb = consts.tile([P, 4, 512], FP16)
        # prefire the Exp table load so it overlaps the input DMA head
        dummy = consts.tile([1, 8], F32)
        nc.vector.memset(dummy, 0.0)
        nc.scalar.activation(out=dummy, in_=dummy,
                             func=mybir.ActivationFunctionType.Exp)

        dram = tc.alloc_tile_pool(name="dram", bufs=1, space="DRAM")
        attd = dram.tile([HPC, N, D], FP16)          # normalized attn out

        big = tc.alloc_tile_pool(name="big", bufs=1)
        qs = big.tile([P, HPC, N], FP16)                  # q^T, heads stacked
        ks = big.tile([P, HPC, N], FP16)                  # k^T
        v_all = big.tile([P, NB, HPC, D + 4], FP16)       # [V | 1] per block
        nc.vector.memset(v_all[:, :, :, D:D + 1], 1.0)

        outT_pool = tc.alloc_tile_pool(name="outT", bufs=1)
        outTs = [outT_pool.tile([P, N], FP16, name=f"outT{t}")
                 for t in range(HPC)]

        # ---------------- attention pools ----------------
        pt_pool = tc.alloc_tile_pool(name="pt_pool", bufs=3)
        att_pool = tc.alloc_tile_pool(name="att_pool", bufs=2)
        rs_pool = tc.alloc_tile_pool(name="rs_pool", bufs=8)
        qk_ps = tc.alloc_tile_pool(name="qk_ps", bufs=2, space="PSUM")
        av_ps = tc.alloc_tile_pool(name="av_ps", bufs=2, space="PSUM")
        # ---- proj-phase pools, on top of the stack (released before oproj)
        wpool = tc.alloc_tile_pool(name="wpool", bufs=1)
        xpool = tc.alloc_tile_pool(name="xpool", bufs=2)
        pj_ps = tc.alloc_tile_pool(name="pj_ps", bufs=2, space="PSUM")

        wq_sb = wpool.tile([P, ET, DC], FP16)
        wk_sb = wpool.tile([P, ET, DC], FP16)
        wv_sb = wpool.tile([P, ET, DC], FP16)
        x_tiles = [None] * NCH

        def load_x(nch):
            # on the SP queue: the Pool queue carries evictions, and this
            # DMA's WAR wait on the old buffer would head-of-line block them
            t = xpool.tile([P, ET, 512], FP16, tag="xchunk",
                           name=f"x_sb{nch}")
            nc.sync.dma_start(
                out=t, in_=xT_r[:, :, nch * 512:(nch + 1) * 512])
            x_tiles[nch] = t

        # two DMA queues (SP + Pool) with wq/x0 split in eo-pairs and issued
        # alternately: DMA transfers serialize globally in trigger order, so
        # arrival must match consumption: wq/x0 interleaved (first qk
        # groups), then wk, wv, mask, and x1 last (needed only at chunk 1)
        x0 = xpool.tile([P, ET, 512], FP16, tag="xchunk", name="x_sb0")
        x_tiles[0] = x0
        nc.sync.dma_start(out=wq_sb[:, 0:1, :], in_=wqT_r[:, 0:1, :])
        nc.gpsimd.dma_start(out=x0[:, 0:1, :], in_=xT_r[:, 0:1, 0:512])
        nc.sync.dma_start(out=wq_sb[:, 1:2, :], in_=wqT_r[:, 1:2, :])
        nc.gpsimd.dma_start(out=x0[:, 1:2, :], in_=xT_r[:, 1:2, 0:512])
        for g in range(1, ET // 2):
            gs = slice(2 * g, 2 * g + 2)
            nc.sync.dma_start(out=wq_sb[:, gs, :], in_=wqT_r[:, gs, :])
            nc.gpsimd.dma_start(out=x0[:, gs, :], in_=xT_r[:, gs, 0:512])
        for g in range(ET // 2):
            gs = slice(2 * g, 2 * g + 2)
            nc.sync.dma_start(out=wk_sb[:, gs, :], in_=wkT_r[:, gs, :])
        QT = ET // 4
        for g in range(4):
            gs = slice(g * QT, (g + 1) * QT)
            nc.gpsimd.dma_start(out=wv_sb[:, gs, :], in_=wvT_r[:, gs, :])
        nc.gpsimd.dma_start(out=mask_sb, in_=maskin.ap())
        x1 = xpool.tile([P, ET, 512], FP16, tag="xchunk", name="x_sb1")
        x_tiles[1] = x1
        nc.gpsimd.dma_start(out=x1[:, :ET // 2, :],
                            in_=xT_r[:, :ET // 2, 512:1024])
        nc.gpsimd.dma_start(out=x1[:, ET // 2:, :],
                            in_=xT_r[:, ET // 2:, 512:1024])

        # PSUM evictions: only DVE and ACT can read PSUM (GPSIMD cannot).
        # ACT is reserved for exp while attention is interleaved, so it only
        # helps during chunk 0 and the out-projection tail.
        _ev = [0]
        _copies = (nc.vector.tensor_copy, nc.scalar.copy)

        def evict(out_ap, in_ap, use_act):
            n = 2 if use_act else 1
            _copies[_ev[0] % n](out=out_ap, in_=in_ap)
            _ev[0] += 1

        # ---------------- proj psum-group closures ----------------
        def proj_groups(nch):
            x_sb = x_tiles[nch]
            nsl = slice(nch * 512, (nch + 1) * 512)
            groups = []

            def qk_group(w_sb, dst, t):
                def run():
                    ps = pj_ps.tile([P, 512], F32, tag="pjps")
                    for et in range(ET):
                        nc.tensor.matmul(
                            ps,
                            lhsT=w_sb[:, et, t * P:(t + 1) * P],
                            rhs=x_sb[:, et, :],
                            start=(et == 0), stop=(et == ET - 1),
                        )
                    evict(dst[:, t, nsl], ps, nch == 0)
                return run

            def v_group(nb):
                def run():
                    ps = pj_ps.tile([P, 512], F32, tag="pjps")
                    for et in range(ET):
                        nc.tensor.matmul(
                            ps,
                            lhsT=x_sb[:, et, nb * P:(nb + 1) * P],
                            rhs=wv_sb[:, et, :],
                            start=(et == 0), stop=(et == ET - 1),
                        )
                    evict(v_all[:, nch * 4 + nb, :, :D],
                          ps.rearrange("p (h d) -> p h d", h=HPC), nch == 0)
                return run

            for w_sb, dst in ((wq_sb, qs), (wk_sb, ks)):
                for t in range(HPC):
                    groups.append(qk_group(w_sb, dst, t))
            for nb in range(4):
                groups.append(v_group(nb))
            return groups

        def proj_groups_c0():
            """Chunk 0 is DMA-arrival-bound: emit quarter-contraction steps
            round-robining all four heads, borrowing the (idle) attention
            qk_ps pair tiles so four psums are in flight and the PE always
            has work per arriving wq/x0 eo-pair."""
            x_sb = x_tiles[0]
            nsl = slice(0, 512)
            state = {}
            groups = []

            NQ = 8  # steps per accumulation: one eo-pair (one DMA piece)

            def qk_step(w_sb, dst, key, t, u, q):
                def run():
                    if q == 0 and u == 0:
                        state[key] = qk_ps.tile([P, 2, 512], F32, tag="qkps",
                                                name=f"c0{key}")
                    ps = state[key]
                    for et in range(2 * q, 2 * q + 2):
                        nc.tensor.matmul(
                            ps[:, u, :],
                            lhsT=w_sb[:, et, t * P:(t + 1) * P],
                            rhs=x_sb[:, et, :],
                            start=(et == 0), stop=(et == ET - 1),
                        )
                    if q == NQ - 1 and u == 1:
                        evict(dst[:, t - 1:t + 1, nsl], ps, True)
                return run

            def v_step(key, nbp, u, q):
                def run():
                    if q == 0 and u == 0:
                        state[key] = qk_ps.tile([P, 2, 512], F32, tag="qkps",
                                                name=f"c0{key}")
                    ps = state[key]
                    nb = 2 * nbp + u
                    for et in range(2 * q, 2 * q + 2):
                        nc.tensor.matmul(
                            ps[:, u, :],
                            lhsT=x_sb[:, et, nb * P:(nb + 1) * P],
                            rhs=wv_sb[:, et, :],
                            start=(et == 0), stop=(et == ET - 1),
                        )
                    if q == NQ - 1 and u == 1:
                        evict(v_all[:, 2 * nbp:2 * nbp + 2, :, :D],
                              ps.rearrange("p g (h d) -> p g h d", h=HPC),
                              True)
                return run

            for w_sb, dst, wn in ((wq_sb, qs, "q"), (wk_sb, ks, "k")):
                for q in range(NQ):
                    for pi in range(2):
                        for u in range(2):
                            groups.append(
                                qk_step(w_sb, dst, wn + str(pi),
                                        2 * pi + u, u, q))
            for q in range(NQ):
                for nbp in range(2):
                    for u in range(2):
                        groups.append(v_step("v" + str(nbp), nbp, u, q))
            return groups

        # ---------------- attention micro-op closures ----------------
        stage_pt = {}

        def qk_ops(h, ci):
            """One op per QK^T pair: 2 matmuls + exp; masks on the last."""
            BJ = 4 * (ci + 1)
            ops = []

            def first_extra(pt):
                # clear regions partial-exp never writes on this rotating
                # buffer (exactly the mask-0 prefix of the last diag pair)
                nc.gpsimd.memset(pt[:, BJ - 2, :256], 0.0)
                nc.gpsimd.memset(pt[:, BJ - 1, :384], 0.0)

            def pair_op(bjp):
                def run():
                    if bjp == 0:
                        pt = pt_pool.tile([P, ET, 512], FP16, tag="pt",
                                          name=f"pt{h}_{ci}")
                        stage_pt[(h, ci)] = pt
                        first_extra(pt)
                    pt = stage_pt[(h, ci)]
                    ps = qk_ps.tile([P, 2, 512], F32, tag="qkps")
                    last_pair = (bjp == BJ // 2 - 1)
                    for u in range(2):
                        bj = 2 * bjp + u
                        rr = bj - (BJ - 4)
                        if rr > 0:
                            nc.tensor.matmul(
                                ps[:, u, rr * P:],
                                lhsT=ks[:, h, bj * P:(bj + 1) * P],
                                rhs=qs[:, h,
                                       ci * 512 + rr * P:(ci + 1) * 512],
                                start=True, stop=True,
                            )
                        else:
                            nc.tensor.matmul(
                                ps[:, u, :],
                                lhsT=ks[:, h, bj * P:(bj + 1) * P],
                                rhs=qs[:, h, ci * 512:(ci + 1) * 512],
                                start=True, stop=True,
                            )
                    if last_pair:
                        # diagonal blocks r=256,384: only cols >= r valid
                        nc.scalar.activation(
                            out=pt[:, 2 * bjp, 256:], in_=ps[:, 0, 256:],
                            func=mybir.ActivationFunctionType.Exp,
                        )
                        nc.scalar.activation(
                            out=pt[:, 2 * bjp + 1, 384:], in_=ps[:, 1, 384:],
                            func=mybir.ActivationFunctionType.Exp,
                        )
                        # causal masks on the diagonal blocks; full-tile so
                        # the zero prefix also clears stale/garbage regions
                        for rr2 in range(4):
                            bj2 = BJ - 4 + rr2
                            nc.vector.tensor_mul(
                                out=pt[:, bj2, :], in0=pt[:, bj2, :],
                                in1=mask_sb[:, rr2, :])
                    else:
                        nc.scalar.activation(
                            out=pt[:, 2 * bjp:2 * bjp + 2, :], in_=ps,
                            func=mybir.ActivationFunctionType.Exp,
                        )
                return run

            for bjp in range(BJ // 2):
                ops.append(pair_op(bjp))
            return ops

        def av_ops(h, ci):
            """One op per A@[V|1] psum; spill+transpose on the last."""
            ops = []

            def ib_op(ib):
                def run():
                    pt = stage_pt[(h, ci)]
                    if ib == 0:
                        att = att_pool.tile([P, 4, D], FP16, tag="atth",
                                            name=f"att{h}_{ci}")
                        stage_pt[(h, ci, "att")] = att
                    att = stage_pt[(h, ci, "att")]
                    nbj = 4 * ci + ib + 1
                    avp = av_ps.tile([P, D + 4], F32, tag="avps")
                    isl = slice(ib * P, (ib + 1) * P)
                    for bj in range(nbj):
                        nc.tensor.matmul(
                            avp[:, :D + 1],
                            lhsT=pt[:, bj, isl],
                            rhs=v_all[:, bj, h, :D + 1],
                            start=(bj == 0), stop=(bj == nbj - 1),
                        )
                    rs = rs_pool.tile([P, 1], F32, tag="rs")
                    nc.vector.reciprocal_approx_fast(
                        out=rs, in_=avp[:, D:D + 1])
                    nc.vector.tensor_scalar_mul(
                        out=att[:, ib, :], in0=avp[:, :D], scalar1=rs)
                    if ib == 3:
                        # spill + transpose right away so the out-projection
                        # isn't gated on the whole head
                        nc.sync.dma_start(
                            out=attd[h, ci * 512:(ci + 1) * 512, :].rearrange(
                                "(io p) d -> p io d", p=P),
                            in_=att)
                        nc.sync.dma_start_transpose(
                            out=outTs[h][:, ci * 512:(ci + 1) * 512],
                            in_=attd[h, ci * 512:(ci + 1) * 512, :])
                return run

            for ib in range(4):
                ops.append(ib_op(ib))
            return ops

        def stage_ops(ci, pending):
            """Micro-ops for all 4 heads of chunk ci; A@V lags one head so
            exp always has at least a head's worth of PE runway. Returns
            (ops, new_pending) with the last head's A@V deferred."""
            ops = []
            for h in range(HPC):
                ops += qk_ops(h, ci)
                if h == 0:
                    ops += pending
                else:
                    ops += av_ops(h - 1, ci)
            return ops, av_ops(HPC - 1, ci)

        # ---------------- interleaved proj + attention ----------------
        pending = []
        micro = []
        for nch in range(NCH):
            if nch >= 1:
                micro, pending = stage_ops(nch - 1, pending)
            groups = proj_groups(nch) if nch else proj_groups_c0()
            ng = len(groups)
            done = 0
            for i, g in enumerate(groups):
                g()
                target = (i + 1) * len(micro) // ng
                while done < target:
                    micro[done]()
                    done += 1
            if nch + 2 < NCH:
                load_x(nch + 2)

        # ---------------- tail: last attention chunk + out-projection -----
        tail_ops, pending = stage_ops(NCH - 1, pending)

        # proj inputs are dead now; reuse their SBUF for the oproj weights
        pj_ps.release()
        xpool.release()
        wpool.release()
        wo_pool = tc.alloc_tile_pool(name="wo_pool", bufs=1)
        wo_sb = wo_pool.tile([P, HPC, E], FP16)
        for ec in range(NCH):
            esl = slice(ec * 512, (ec + 1) * 512)
            nc.sync.dma_start(out=wo_sb[:, :, esl], in_=woT_r[:, :, esl])

        op_ps = tc.alloc_tile_pool(name="op_ps", bufs=2, space="PSUM")
        op_ev = tc.alloc_tile_pool(name="op_ev", bufs=3)
        _oev = [0]

        def oproj_group(nb, ec, ostage):
            ps = op_ps.tile([P, 512], F32, tag="opps")
            for t in range(HPC):
                nc.tensor.matmul(
                    ps,
                    lhsT=outTs[t][:, nb * P:(nb + 1) * P],
                    rhs=wo_sb[:, t, ec * 512:(ec + 1) * 512],
                    start=(t == 0), stop=(t == HPC - 1),
                )
            _copies[_oev[0] % 2](out=ostage[:, ec, :], in_=ps)
            _oev[0] += 1

        _dmas2 = (nc.sync, nc.scalar)

        def oproj_final_ec(nb, ec, ostage, ps):
            # very last psum group: halve everything so the critical chain
            # after the final matmul is one small evict + two parallel DMAs
            orow = out.ap()[nb * P:(nb + 1) * P, :]
            for half in range(2):
                csl = slice(half * 256, (half + 1) * 256)
                for t in range(HPC):
                    nc.tensor.matmul(
                        ps[:, csl],
                        lhsT=outTs[t][:, nb * P:(nb + 1) * P],
                        rhs=wo_sb[:, t, ec * 512 + half * 256:
                                  ec * 512 + (half + 1) * 256],
                        start=(t == 0), stop=(t == HPC - 1),
                    )
                _copies[half](out=ostage[:, ec, csl], in_=ps[:, csl])
                _dmas2[half].dma_start(
                    out=orow[:, ec * 512 + half * 256:
                             ec * 512 + (half + 1) * 256],
                    in_=ostage[:, ec, csl])

        def oproj_nb(nb, split_dma=False):
            ostage = op_ev.tile([P, NCH, 512], FP16, tag="opev",
                                name=f"ostage{nb}")
            # the last block reuses a freed attention psum pair for its last
            # two groups, so it never waits on op_ps buffer rotation
            fin_ps = (qk_ps.tile([P, 2, 512], F32, tag="qkps",
                                 name="fin_ps")
                      if split_dma and nb == NB - 1 else None)
            for ec in range(NCH):
                eng = _oev[0] % 2
                if fin_ps is not None and ec == NCH - 2:
                    ps = fin_ps[:, 0, :]
                    for t in range(HPC):
                        nc.tensor.matmul(
                            ps,
                            lhsT=outTs[t][:, nb * P:(nb + 1) * P],
                            rhs=wo_sb[:, t, ec * 512:(ec + 1) * 512],
                            start=(t == 0), stop=(t == HPC - 1),
                        )
                    _copies[eng](out=ostage[:, ec, :], in_=ps)
                    _oev[0] += 1
                    _dmas2[eng].dma_start(
                        out=out.ap()[nb * P:(nb + 1) * P,
                                     ec * 512:(ec + 1) * 512],
                        in_=ostage[:, ec, :])
                    continue
                if fin_ps is not None and ec == NCH - 1:
                    oproj_final_ec(nb, ec, ostage, fin_ps[:, 1, :])
                    continue
                oproj_group(nb, ec, ostage)
                if split_dma:
                    # final blocks: stream per-ec pieces from the queue of
                    # the engine that evicted them, so the kernel tail is
                    # several concurrent small DMAs, not one 512KB one
                    _dmas2[eng].dma_start(
                        out=out.ap()[nb * P:(nb + 1) * P,
                                     ec * 512:(ec + 1) * 512],
                        in_=ostage[:, ec, :])
            if not split_dma:
                nc.sync.dma_start(
                    out=out.ap()[nb * P:(nb + 1) * P, :], in_=ostage)

        # pace the last attention chunk across oproj blocks 0..7 (only
        # chunks ci<=2 feed them, all transposed already) at ec-group
        # granularity so qk pairs never burst past the 2-deep PSUM window;
        # run the first few attention ops up front to hide the wo_sb DMA
        done = 0
        while done < len(tail_ops) // 9:
            tail_ops[done]()
            done += 1
        n_ec = 8 * NCH
        ostages = {}
        for i in range(n_ec):
            nb, ec = divmod(i, NCH)
            if ec == 0:
                ostages[nb] = op_ev.tile([P, NCH, 512], FP16, tag="opev",
                                         name=f"ostage{nb}")
            oproj_group(nb, ec, ostages[nb])
            if ec == NCH - 1:
                nc.sync.dma_start(
                    out=out.ap()[nb * P:(nb + 1) * P, :], in_=ostages[nb])
            target = len(tail_ops) // 9 + (i + 1) * (
                len(tail_ops) - len(tail_ops) // 9) // n_ec
            while done < target:
                tail_ops[done]()
                done += 1
        while done < len(tail_ops):
            tail_ops[done]()
            done += 1
        oproj_nb(8)
        oproj_nb(9)
        for op in pending:   # A@V of the very last stage (exp has had runway)
            op()
        for nb in range(10, NB):
            oproj_nb(nb, split_dma=(nb >= NB - 2))

        for pool in (op_ev, op_ps, wo_pool, av_ps, qk_ps, rs_pool, att_pool,
                     pt_pool, outT_pool, big, dram, consts):
            pool.release()

    nc.compile()
    return nc


def make_in_maps(x, Wq, Wkv, Wout):
    x = np.asarray(x, dtype=np.float32)
    Wq = np.asarray(Wq, dtype=np.float32)
    Wkv = np.asarray(Wkv, dtype=np.float32)
    Wout = np.asarray(Wout, dtype=np.float32)
    scale = np.float32(D ** -0.5)

    # causal masks for the 4 diagonal offsets
    jj = np.arange(P)[:, None]
    ii = np.arange(512)[None, :]
    mask = np.zeros((P, 4, 512), dtype=np.float16)
    for rr in range(4):
        mask[:, rr, :] = (ii >= jj + rr * P).astype(np.float16)

    xT = [np.ascontiguousarray(x[b].T).astype(np.float16) for b in range(B)]
    in_maps = []
    for c in range(NCORES):
        b, hg = divmod(c, 4)
        sl = slice(hg * DC, (hg + 1) * DC)
        in_maps.append({
            "xT": xT[b],
            "wqT": (np.ascontiguousarray(Wq[sl, :].T) * scale).astype(np.float16),
            "wkT": np.ascontiguousarray(Wkv[sl, :].T).astype(np.float16),
            "wvT": np.ascontiguousarray(Wkv[E + sl.start:E + sl.stop, :].T).astype(np.float16),
            "woT": np.ascontiguousarray(Wout[:, sl].T).astype(np.float16),
            "maskin": mask,
        })
    return in_maps


_NC_CACHE = []


def _get_nc():
    if not _NC_CACHE:
        _NC_CACHE.append(build_nc())
    return _NC_CACHE[0]


def _run(in_maps):
    nc = _get_nc()
    return run_bass_kernel_spmd(nc, in_maps, core_ids=list(range(NCORES)))


def kernel(x, Wq, Wkv, Wout):
    in_maps = make_in_maps(x, Wq, Wkv, Wout)
    res = _run(in_maps)
    out = np.zeros((B, N, E), dtype=np.float32)
    for c in range(NCORES):
        out[c // 4] += res.results[c]["out"].astype(np.float32)
    return out


if __name__ == "__main__":
    t0 = time.time()
    _get_nc()
    print(f"build+compile: {time.time() - t0:.1f}s")
